# revision 17
# baseline (speedup 1.0000x reference)
"""Trainium2 Bass kernel for a dense transformer decoder block.

Strategy: pure data parallelism -- batch dim (8) sharded 1:1 onto the 8
NeuronCores; each core runs the full decoder block on its [1024, 1024]
slice. No collectives needed.

Per-core dataflow (T=1024, D=1024, H=16, hs=64, Dff=4096):
  - LN1 in natural [token_p, d_f] layout (free-dim reductions), output
    transposed via PE into hT [d_p, token_f] fp16.
  - QKV: qT/kT = W.T-side matmuls (lhsT=W chunk, rhs=hT) giving
    transposed activations; v computed in natural layout
    (lhsT=hT chunk, rhs=Wv) and packed into v_aug with a ones column
    per head so the AV matmul emits softmax denominators for free.
  - Attention entirely in "scores transposed" [tk_p, tq_f] layout:
    exp without max-subtraction (logits bounded ~|0.9|), causal blocks
    skipped, diagonal blocks masked post-exp with a triangular mask.
    AV: lhsT = v_aug[:, i, h, :] (M=65: 64 outputs + denominator row).
  - Wo/FFN as standard K-accumulated matmuls; fp16 operands with fp32
    PSUM accumulation everywhere (full PE rate, ~1e-3 relative error).
"""

from contextlib import ExitStack

import numpy as np

import concourse.bacc as bacc
import concourse.bass as bass
import concourse.mybir as mybir
import concourse.tile as tile

T = 1024
D = 1024
H = 16
HS = 64
DFF = 4096
P = 128
NT = T // P
ND = D // P
NG = DFF // P
EPS = 1e-5
SCALE = 1.0 / 32.0  # 1/sqrt(D)
N_CORES = 8

F32 = mybir.dt.float32
F16 = mybir.dt.float16

# Compacted causal E-layout: chunk i stores columns [128*i, 1024).
E_OFF = [0] * NT
for _i in range(1, NT):
    E_OFF[_i] = E_OFF[_i - 1] + (T - P * (_i - 1))
E_TOT = E_OFF[-1] + (T - P * (NT - 1))  # 4608


def _ln_chunk(nc, pool, x_ap, eps_tile, out_ap):
    """Plain (x - mean) * rstd over the free dim of a [128, D] chunk.

    The LN gain/bias are folded into the consuming weight matrices and
    bias rows on the host (see prepare_base_inputs), so no g/b here.
    """
    stats = pool.tile([P, 2, 6], F32, tag="ln_stats", name="stats")
    mv = pool.tile([P, 2], F32, tag="ln_mv", name="mv")
    xg = x_ap.rearrange("p (n f) -> p n f", f=512)
    for sg in range(2):
        nc.vector.bn_stats(out=stats[:, sg, :], in_=xg[:, sg, :])
    nc.vector.bn_aggr(out=mv, in_=stats)
    rstd = pool.tile([P, 1], F32, tag="ln_rstd", name="rstd")
    nc.scalar.activation(out=rstd, in_=mv[:, 1:2],
                         func=mybir.ActivationFunctionType.Sqrt,
                         bias=eps_tile, scale=1.0)
    nc.vector.reciprocal(out=rstd, in_=rstd)
    nc.vector.tensor_scalar(out=out_ap, in0=x_ap, scalar1=mv[:, 0:1],
                            scalar2=rstd, op0=mybir.AluOpType.subtract,
                            op1=mybir.AluOpType.mult)


def _transpose_into(nc, psum_pool, src_ap, dst_tile, dst_col0, identity):
    """PE-transpose [128, 128] fp16 chunks of src_ap [128, D] into
    dst_tile[:, dc, dst_col0:dst_col0+128]."""
    for dc in range(ND):
        pt = psum_pool.tile([P, P], F16, tag="tr", name="pt")
        nc.tensor.transpose(pt, src_ap[:, dc * P:(dc + 1) * P], identity)
        nc.vector.tensor_copy(out=dst_tile[:, dc, dst_col0:dst_col0 + P],
                              in_=pt)


def build(repeat: int = 0):
    nc = bacc.Bacc()
    dram = {}
    dram["x"] = nc.dram_tensor("x", [T, D], F32, kind="ExternalInput")
    # Wq/Wk pre-packed on host as [m, p, c, mcol] so the per-head-pair DMA
    # reads 2KB contiguous lines; W1 likewise per g-chunk.
    dram["Wq"] = nc.dram_tensor("Wq", [ND, P, ND, P], F16,
                                kind="ExternalInput")
    dram["Wk"] = nc.dram_tensor("Wk", [ND, P, ND, P], F16,
                                kind="ExternalInput")
    dram["Wv"] = nc.dram_tensor("Wv", [D, D], F16, kind="ExternalInput")
    dram["Wo"] = nc.dram_tensor("Wo", [D, D], F16, kind="ExternalInput")
    dram["W1"] = nc.dram_tensor("W1", [NG, P, ND, P], F16,
                                kind="ExternalInput")
    dram["W2"] = nc.dram_tensor("W2", [DFF, D], F16, kind="ExternalInput")
    dram["bo"] = nc.dram_tensor("bo", [D], F32, kind="ExternalInput")
    dram["b1"] = nc.dram_tensor("b1", [DFF], F32, kind="ExternalInput")
    dram["b2"] = nc.dram_tensor("b2", [D], F32, kind="ExternalInput")
    for b in ("bq", "bk", "bv"):
        dram[b] = nc.dram_tensor(b, [D], F32, kind="ExternalInput")
    dram["out"] = nc.dram_tensor("out", [T, D], F32, kind="ExternalOutput")

    with tile.TileContext(nc) as tc:
        if repeat > 0:
            with tc.For_i(0, repeat, 1):
                _body(nc, tc, dram)
        else:
            _body(nc, tc, dram)
    nc.finalize()
    return nc


def _body(nc, tc, dram):
    AF = mybir.ActivationFunctionType
    x_d = dram["x"]
    out_d = dram["out"]

    with ExitStack() as body_es:
        consts = body_es.enter_context(tc.tile_pool(name="consts", bufs=1))
        # --- small constants (live for whole body) ---
        identity = consts.tile([P, P], F16)
        nc.gpsimd.memset(identity, 0.0)
        nc.gpsimd.affine_select(out=identity, in_=identity,
                                compare_op=mybir.AluOpType.not_equal,
                                fill=1.0, base=0, pattern=[[-1, P]],
                                channel_multiplier=1)
        # tri[x, y] = 1 where y >= x else 0   (valid tk <= tq)
        tri = consts.tile([P, P], F16)
        nc.gpsimd.memset(tri, 1.0)
        nc.gpsimd.affine_select(out=tri, in_=tri,
                                compare_op=mybir.AluOpType.is_ge,
                                fill=0.0, base=0, pattern=[[1, P]],
                                channel_multiplier=-1)
        eps_tile = consts.tile([P, 1], F32)
        nc.vector.memset(eps_tile, EPS)

        # Out-of-order pool lifetimes, closed manually:
        es_hT = ExitStack()       # phase A .. C
        es_vaug = ExitStack()     # phase B .. C
        es_attT = ExitStack()     # phase C .. D
        es_out1 = ExitStack()     # phase D .. E

        hTp = es_hT.enter_context(tc.tile_pool(name="hTp", bufs=1))
        hT = hTp.tile([P, ND, T], F16)

        # ---------------- Phase A: LN1 + transpose ----------------
        with tc.tile_pool(name="ln1", bufs=3) as lnp, \
             tc.tile_pool(name="pt_a", bufs=4, space="PSUM") as ptp:
            for tcn in range(NT):
                x_t = lnp.tile([P, D], F32, tag="x", name="x_t")
                nc.sync.dma_start(out=x_t, in_=x_d[tcn * P:(tcn + 1) * P, :])
                h_t = lnp.tile([P, D], F16, tag="h", name="h_t")
                _ln_chunk(nc, lnp, x_t, eps_tile, h_t)
                _transpose_into(nc, ptp, h_t, hT, tcn * P, identity)

        # ---------------- Phase B: v projection -> v_aug ----------------
        vaugp = es_vaug.enter_context(tc.tile_pool(name="vaugp", bufs=1))
        v_aug = vaugp.tile([P, NT, H, HS + 1], F16)
        with tc.tile_pool(name="wv", bufs=1) as wvp, \
             tc.tile_pool(name="ps_b", bufs=4, space="PSUM") as psb:
            wv_sb = wvp.tile([P, ND, D], F16)
            for dc in range(ND):
                nc.sync.dma_start(out=wv_sb[:, dc, :],
                                  in_=dram["Wv"][dc * P:(dc + 1) * P, :])
            bv_b = wvp.tile([P, D], F32)
            nc.sync.dma_start(out=bv_b,
                              in_=dram["bv"].ap().partition_broadcast(P))
            nc.vector.memset(v_aug[:, :, :, HS:HS + 1], 1.0)
            for tcn in range(NT):
                for ns in range(2):
                    pv = psb.tile([P, 512], F32, tag="pv", name="pv")
                    for dc in range(ND):
                        nc.tensor.matmul(pv,
                                         lhsT=hT[:, dc, tcn * P:(tcn + 1) * P],
                                         rhs=wv_sb[:, dc, ns * 512:(ns + 1) * 512],
                                         start=(dc == 0), stop=(dc == ND - 1))
                    bv_ap = bv_b[:, ns * 512:(ns + 1) * 512].rearrange(
                        "p (h s) -> p h s", s=HS)
                    nc.vector.tensor_add(
                        out=v_aug[:, tcn, ns * 8:(ns + 1) * 8, 0:HS],
                        in0=pv.rearrange("p (h s) -> p h s", s=HS),
                        in1=bv_ap)

        # ---------------- Phase C: attention per head-pair ----------------
        attTp = es_attT.enter_context(
            tc.tile_pool(name="attTp", bufs=1, side="right"))
        attT = attTp.tile([P, ND, T], F16)
        with tc.tile_pool(name="qk", bufs=2) as qkp, \
             tc.tile_pool(name="e", bufs=1) as ep, \
             tc.tile_pool(name="attn_sm", bufs=2) as smp, \
             tc.tile_pool(name="ps_s", bufs=4, space="PSUM") as pss, \
             tc.tile_pool(name="ps_av", bufs=2, space="PSUM") as psav:
            e_tiles = [ep.tile([P, E_TOT], F16, tag=f"e{i}", name=f"e{i}")
                       for i in range(2)]
            bq_sb = ep.tile([P, ND], F32, name="bq_sb")
            nc.sync.dma_start(out=bq_sb,
                              in_=dram["bq"].ap().rearrange("(m p) -> p m",
                                                            p=P))
            bk_sb = ep.tile([P, ND], F32, name="bk_sb")
            nc.sync.dma_start(out=bk_sb,
                              in_=dram["bk"].ap().rearrange("(m p) -> p m",
                                                            p=P))
            for m in range(ND):  # head pair m -> heads 2m, 2m+1
                wq_m = qkp.tile([P, ND, P], F16, tag="wqm", name="wq_m")
                nc.sync.dma_start(out=wq_m, in_=dram["Wq"][m])
                wk_m = qkp.tile([P, ND, P], F16, tag="wkm", name="wk_m")
                nc.sync.dma_start(out=wk_m, in_=dram["Wk"][m])
                qT_m = qkp.tile([P, T], F16, tag="qTm", name="qT_m")
                kT_m = qkp.tile([P, T], F16, tag="kTm", name="kT_m")
                for dst, w_m, b_sb in ((qT_m, wq_m, bq_sb),
                                       (kT_m, wk_m, bk_sb)):
                    for ns in range(2):
                        pq = pss.tile([P, 512], F32, tag="ps", name="pq")
                        for dc in range(ND):
                            nc.tensor.matmul(
                                pq, lhsT=w_m[:, dc, :],
                                rhs=hT[:, dc, ns * 512:(ns + 1) * 512],
                                start=(dc == 0), stop=(dc == ND - 1))
                        nc.vector.tensor_scalar_add(
                            out=dst[:, ns * 512:(ns + 1) * 512], in0=pq,
                            scalar1=b_sb[:, m:m + 1])

                # scores + exp (+ diagonal causal mask). Per (i, head) the
                # kT block stays stationary across the tq slices.
                for i in range(NT):
                    for hs_sel in range(2):
                        pb = hs_sel * HS
                        qs = i * P
                        while qs < T:
                            qw = min(512, T - qs)
                            ps_ = pss.tile([P, 512], F32, tag="ps", name="ps_")
                            nc.tensor.matmul(
                                ps_[:, 0:qw],
                                lhsT=kT_m[pb:pb + HS, i * P:(i + 1) * P],
                                rhs=qT_m[pb:pb + HS, qs:qs + qw],
                                start=True, stop=True)
                            ec = e_tiles[hs_sel][:, E_OFF[i] + qs - i * P:
                                                 E_OFF[i] + qs - i * P + qw]
                            nc.scalar.activation(out=ec, in_=ps_[:, 0:qw],
                                                 func=AF.Exp, scale=SCALE)
                            qs += qw
                        dg = e_tiles[hs_sel][:, E_OFF[i]:E_OFF[i] + P]
                        nc.vector.tensor_mul(out=dg, in0=dg, in1=tri)

                # AV + normalize; i-outer so both tq-slice accumulation
                # groups reuse the stationary v_aug block per chunk.
                for hs_sel in range(2):
                    h_glob = 2 * m + hs_sel
                    pavs = [psav.tile([HS + 1, 512], F32, tag=f"pav{s}",
                                      name=f"pav{s}") for s in range(2)]
                    for i in range(NT):
                        for s in range(2):
                            if i * P >= (s + 1) * 512:
                                continue
                            i_last = min(NT - 1, ((s + 1) * 512 - 1) // P)
                            sub_lo = max(i * P, s * 512)
                            width = (s + 1) * 512 - sub_lo
                            off = sub_lo - s * 512
                            e_ap = e_tiles[hs_sel][
                                :, E_OFF[i] + sub_lo - i * P:
                                E_OFF[i] + sub_lo - i * P + width]
                            nc.tensor.matmul(
                                pavs[s][:, off:off + width],
                                lhsT=v_aug[:, i, h_glob, :],
                                rhs=e_ap,
                                start=(i == 0), stop=(i == i_last))
                    for s in range(2):
                        pav = pavs[s]
                        recip = smp.tile([1, 512], F32, tag="recip",
                                         name="recip")
                        nc.vector.reciprocal(out=recip,
                                             in_=pav[HS:HS + 1, :])
                        bcast = smp.tile([HS, 512], F32, tag="bcast",
                                         name="bcast")
                        nc.gpsimd.partition_broadcast(out_ap=bcast,
                                                      in_ap=recip,
                                                      channels=HS)
                        p0 = hs_sel * HS
                        nc.vector.tensor_mul(
                            out=attT[p0:p0 + HS, m, s * 512:(s + 1) * 512],
                            in0=pav[0:HS, :], in1=bcast)
        es_vaug.close()
        es_hT.close()

        # ---------------- Phase D: Wo + residual + LN2 ----------------
        out1p = es_out1.enter_context(tc.tile_pool(name="out1p", bufs=1))
        out1 = out1p.tile([P, NT, D], F16)
        h2T = out1p.tile([P, ND, T], F16)
        with tc.tile_pool(name="wo", bufs=1) as wop, \
             tc.tile_pool(name="ln2", bufs=2) as ln2p, \
             tc.tile_pool(name="ps_d", bufs=4, space="PSUM") as psd, \
             tc.tile_pool(name="pt_d", bufs=4, space="PSUM") as ptd:
            bo_b = wop.tile([P, D], F32)
            nc.sync.dma_start(out=bo_b,
                              in_=dram["bo"].ap().partition_broadcast(P))
            wo_sb = wop.tile([P, ND, D], F16)
            for dc in range(ND):
                nc.sync.dma_start(out=wo_sb[:, dc, :],
                                  in_=dram["Wo"][dc * P:(dc + 1) * P, :])
            for tcn in range(NT):
                x_t = ln2p.tile([P, D], F32, tag="x2", name="x_t2")
                nc.sync.dma_start(out=x_t, in_=x_d[tcn * P:(tcn + 1) * P, :])
                for ns in range(2):
                    po = psd.tile([P, 512], F32, tag="po", name="po")
                    for dc in range(ND):
                        nc.tensor.matmul(
                            po, lhsT=attT[:, dc, tcn * P:(tcn + 1) * P],
                            rhs=wo_sb[:, dc, ns * 512:(ns + 1) * 512],
                            start=(dc == 0), stop=(dc == ND - 1))
                    stage = ln2p.tile([P, 512], F32, tag="stage", name="stage")
                    nc.vector.tensor_add(out=stage, in0=po,
                                         in1=x_t[:, ns * 512:(ns + 1) * 512])
                    nc.vector.tensor_add(
                        out=out1[:, tcn, ns * 512:(ns + 1) * 512],
                        in0=stage, in1=bo_b[:, ns * 512:(ns + 1) * 512])
                h2_t = ln2p.tile([P, D], F16, tag="h2", name="h2_t")
                _ln_chunk(nc, ln2p, out1[:, tcn, :], eps_tile, h2_t)
                _transpose_into(nc, ptd, h2_t, h2T, tcn * P, identity)
        es_attT.close()

        # ---------------- Phase E: FFN ----------------
        with tc.tile_pool(name="w2", bufs=1) as w2p, \
             tc.tile_pool(name="fTp", bufs=1) as fTp, \
             tc.tile_pool(name="w1", bufs=3) as w1p, \
             tc.tile_pool(name="ffn_out", bufs=3) as fop, \
             tc.tile_pool(name="ps_f", bufs=2, space="PSUM") as psf, \
             tc.tile_pool(name="ps_o2", bufs=2, space="PSUM") as pso:
            b1_sb = w2p.tile([P, NG], F32)
            nc.sync.dma_start(out=b1_sb,
                              in_=dram["b1"].ap().rearrange("(g p) -> p g",
                                                            p=P))
            b2_b = w2p.tile([P, D], F32)
            nc.sync.dma_start(out=b2_b,
                              in_=dram["b2"].ap().partition_broadcast(P))
            w2_sb = w2p.tile([P, NG, D], F16)
            for g in range(NG):
                nc.sync.dma_start(out=w2_sb[:, g, :],
                                  in_=dram["W2"][g * P:(g + 1) * P, :])
            fT = fTp.tile([P, NG, T], F16)
            for g in range(NG):
                w1_g = w1p.tile([P, ND, P], F16, tag="w1g", name="w1_g")
                nc.sync.dma_start(out=w1_g, in_=dram["W1"][g])
                for th in range(2):
                    pf = psf.tile([P, 512], F32, tag="pf", name="pf")
                    for dc in range(ND):
                        nc.tensor.matmul(
                            pf, lhsT=w1_g[:, dc, :],
                            rhs=h2T[:, dc, th * 512:(th + 1) * 512],
                            start=(dc == 0), stop=(dc == ND - 1))
                    nc.scalar.activation(
                        out=fT[:, g, th * 512:(th + 1) * 512], in_=pf,
                        func=AF.Relu, bias=b1_sb[:, g:g + 1], scale=1.0)
            for tcn in range(NT):
                for js in range(2):
                    po2 = pso.tile([P, 512], F32, tag="po2", name="po2")
                    for g in range(NG):
                        nc.tensor.matmul(
                            po2, lhsT=fT[:, g, tcn * P:(tcn + 1) * P],
                            rhs=w2_sb[:, g, js * 512:(js + 1) * 512],
                            start=(g == 0), stop=(g == NG - 1))
                    ofin = fop.tile([P, 512], F32, tag="ofin", name="ofin")
                    nc.vector.tensor_add(
                        out=ofin, in0=po2,
                        in1=out1[:, tcn, js * 512:(js + 1) * 512])
                    nc.vector.tensor_add(
                        out=ofin, in0=ofin,
                        in1=b2_b[:, js * 512:(js + 1) * 512])
                    nc.sync.dma_start(
                        out=out_d[tcn * P:(tcn + 1) * P,
                                  js * 512:(js + 1) * 512],
                        in_=ofin)
        es_out1.close()


_NC_CACHE = {}


def _get_nc():
    if "nc" not in _NC_CACHE:
        _NC_CACHE["nc"] = build(0)
    return _NC_CACHE["nc"]


def prepare_base_inputs(inputs):
    """Host-side input transform (shared by all cores).

    - Fold LN gains into the consuming weight matrices (Wq/Wk/Wv get
      ln1_g row-scaling; W1 gets ln2_g) and LN biases into bias rows
      (bq/bk/bv = ln1_b @ W; b1 += ln2_b @ W1). Exact algebra: with
      h = h0*g + b, h@W = h0@(g*W) + b@W.
    - Cast matmul weights to fp16.
    - Pre-pack Wq/Wk per head-pair column block and W1 per dff chunk so
      the device DMAs read 2KB-contiguous lines.
    """
    f32 = lambda k: np.asarray(inputs[k], np.float64)
    g1 = f32("ln1_g")[:, None]
    b1v = f32("ln1_b")
    g2 = f32("ln2_g")[:, None]
    b2v = f32("ln2_b")
    Wq, Wk, Wv, Wo = f32("Wq"), f32("Wk"), f32("Wv"), f32("Wo")
    W1, W2 = f32("W1"), f32("W2")

    def pack_cols(w, n_out_chunks):
        # [D, C*P] -> [C, P(row-sub), D//P(row-chunk), P(col)]
        d = w.shape[0]
        return np.ascontiguousarray(
            w.reshape(d // P, P, n_out_chunks, P).transpose(2, 1, 0, 3)
            .astype(np.float16))

    base = {
        "Wq": pack_cols(g1 * Wq, ND),
        "Wk": pack_cols(g1 * Wk, ND),
        "Wv": np.ascontiguousarray((g1 * Wv).astype(np.float16)),
        "Wo": np.ascontiguousarray(Wo.astype(np.float16)),
        "W1": pack_cols(g2 * W1, NG),
        "W2": np.ascontiguousarray(W2.astype(np.float16)),
        "bq": (b1v @ Wq).astype(np.float32),
        "bk": (b1v @ Wk).astype(np.float32),
        "bv": (b1v @ Wv).astype(np.float32),
        "b1": (f32("b1") + b2v @ W1).astype(np.float32),
        "bo": np.asarray(inputs["bo"], np.float32),
        "b2": np.asarray(inputs["b2"], np.float32),
    }
    return base


def kernel(**inputs):
    from concourse.bass_utils import run_bass_kernel_spmd

    nc = _get_nc()
    x = np.ascontiguousarray(np.asarray(inputs["x"], dtype=np.float32))
    base = prepare_base_inputs(inputs)
    in_maps = [dict(x=np.ascontiguousarray(x[c]), **base)
               for c in range(N_CORES)]
    res = run_bass_kernel_spmd(nc, in_maps, list(range(N_CORES)))
    return np.stack([res.results[c]["out"] for c in range(N_CORES)], axis=0)


# revision 34
# speedup vs baseline: 2.0115x; 2.0115x over previous
"""Trainium2 Bass kernel for a dense transformer decoder block.

Strategy: pure data parallelism -- batch dim (8) sharded 1:1 onto the 8
NeuronCores; each core runs the full decoder block on its [1024, 1024]
slice. No collectives needed.

Per-core dataflow (T=1024, D=1024, H=16, hs=64, Dff=4096):
  - LN1 in natural [token_p, d_f] layout (free-dim reductions), output
    transposed via PE into hT [d_p, token_f] fp16.
  - QKV: qT/kT = W.T-side matmuls (lhsT=W chunk, rhs=hT) giving
    transposed activations; v computed in natural layout
    (lhsT=hT chunk, rhs=Wv) and packed into v_aug with a ones column
    per head so the AV matmul emits softmax denominators for free.
  - Attention entirely in "scores transposed" [tk_p, tq_f] layout:
    exp without max-subtraction (logits bounded ~|0.9|), causal blocks
    skipped, diagonal blocks masked post-exp with a triangular mask.
    AV: lhsT = v_aug[:, i, h, :] (M=65: 64 outputs + denominator row).
  - Wo/FFN as standard K-accumulated matmuls; fp16 operands with fp32
    PSUM accumulation everywhere (full PE rate, ~1e-3 relative error).
"""

from contextlib import ExitStack

import numpy as np

import concourse.bacc as bacc
import concourse.bass as bass
import concourse.mybir as mybir
import concourse.tile as tile

T = 1024
D = 1024
H = 16
HS = 64
DFF = 4096
P = 128
NT = T // P
ND = D // P
NG = DFF // P
EPS = 1e-5
SCALE = 1.0 / 32.0  # 1/sqrt(D)
N_CORES = 8

F32 = mybir.dt.float32
F16 = mybir.dt.float16

# Compacted causal E-layout: chunk i stores its valid columns
# [128*i, 1024), i.e. span 1024-128*i. Chunks are stored in E_ORDER so
# that each E_GROUP (chunks sharing one score-PSUM tile, total span
# <= 1024) is contiguous -- one exp op then covers the whole group.
E_GROUPS = [[0], [1], [2, 7], [3, 6], [4, 5]]
E_ORDER = [i for grp in E_GROUPS for i in grp]
E_OFF = [0] * NT
_off = 0
for _i in E_ORDER:
    E_OFF[_i] = _off
    _off += T - P * _i
E_TOT = _off  # 4608


def _ln_chunk(nc, pool, x_ap, eps_tile, out_ap):
    """Plain (x - mean) * rstd over the free dim of a [128, D] chunk.

    The LN gain/bias are folded into the consuming weight matrices and
    bias rows on the host (see prepare_base_inputs), so no g/b here.
    """
    stats = pool.tile([P, 2, 6], F32, tag="ln_stats", name="stats")
    mv = pool.tile([P, 2], F32, tag="ln_mv", name="mv")
    xg = x_ap.rearrange("p (n f) -> p n f", f=512)
    for sg in range(2):
        nc.vector.bn_stats(out=stats[:, sg, :], in_=xg[:, sg, :])
    nc.vector.bn_aggr(out=mv, in_=stats)
    rstd = pool.tile([P, 1], F32, tag="ln_rstd", name="rstd")
    nc.scalar.activation(out=rstd, in_=mv[:, 1:2],
                         func=mybir.ActivationFunctionType.Sqrt,
                         bias=eps_tile, scale=1.0)
    nc.vector.reciprocal(out=rstd, in_=rstd)
    nc.vector.tensor_scalar(out=out_ap, in0=x_ap, scalar1=mv[:, 0:1],
                            scalar2=rstd, op0=mybir.AluOpType.subtract,
                            op1=mybir.AluOpType.mult)


def _transpose_into(nc, psum_pool, src_ap, dst_tile, dst_col0, identity):
    """PE-transpose [128, 128] fp16 chunks of src_ap [128, D] into
    dst_tile[:, dc, dst_col0:dst_col0+128]."""
    for dc in range(ND):
        pt = psum_pool.tile([P, P], F16, tag="tr", name="pt")
        nc.tensor.transpose(pt, src_ap[:, dc * P:(dc + 1) * P], identity)
        nc.vector.tensor_copy(out=dst_tile[:, dc, dst_col0:dst_col0 + P],
                              in_=pt)


def build(repeat: int = 0):
    nc = bacc.Bacc()
    dram = {}
    dram["x"] = nc.dram_tensor("x", [T, D], F32, kind="ExternalInput")
    # Wq/Wk pre-packed on host as [m, p, c, mcol] so the per-head-pair DMA
    # reads 2KB contiguous lines; W1 likewise per g-chunk.
    dram["Wq"] = nc.dram_tensor("Wq", [ND, P, ND, P], F16,
                                kind="ExternalInput")
    dram["Wk"] = nc.dram_tensor("Wk", [ND, P, ND, P], F16,
                                kind="ExternalInput")
    dram["Wv"] = nc.dram_tensor("Wv", [D, D], F16, kind="ExternalInput")
    dram["Wo"] = nc.dram_tensor("Wo", [D, D], F16, kind="ExternalInput")
    dram["W1"] = nc.dram_tensor("W1", [NG, P, ND, P], F16,
                                kind="ExternalInput")
    dram["W2"] = nc.dram_tensor("W2", [DFF, D], F16, kind="ExternalInput")
    dram["bo"] = nc.dram_tensor("bo", [D], F32, kind="ExternalInput")
    # b1/bq/bk are host-packed to [P, chunks] so the DMA reads contiguous
    # per-partition rows instead of thousands of 4-byte gathers.
    dram["b1"] = nc.dram_tensor("b1", [P, NG], F32, kind="ExternalInput")
    dram["b2"] = nc.dram_tensor("b2", [D], F32, kind="ExternalInput")
    for b in ("bq", "bk"):
        dram[b] = nc.dram_tensor(b, [P, ND], F32, kind="ExternalInput")
    dram["bv"] = nc.dram_tensor("bv", [D], F32, kind="ExternalInput")
    dram["out"] = nc.dram_tensor("out", [T, D], F32, kind="ExternalOutput")

    with tile.TileContext(nc) as tc:
        if repeat > 0:
            with tc.For_i(0, repeat, 1):
                _body(nc, tc, dram)
        else:
            _body(nc, tc, dram)
    nc.finalize()
    return nc


def _body(nc, tc, dram):
    AF = mybir.ActivationFunctionType
    x_d = dram["x"]
    out_d = dram["out"]

    with ExitStack() as body_es:
        consts = body_es.enter_context(tc.tile_pool(name="consts", bufs=1))
        # --- small constants (live for whole body) ---
        identity = consts.tile([P, P], F16)
        nc.gpsimd.memset(identity, 0.0)
        nc.gpsimd.affine_select(out=identity, in_=identity,
                                compare_op=mybir.AluOpType.not_equal,
                                fill=1.0, base=0, pattern=[[-1, P]],
                                channel_multiplier=1)
        # tri[x, y] = 1 where y >= x else 0   (valid tk <= tq)
        tri = consts.tile([P, P], F16)
        nc.gpsimd.memset(tri, 1.0)
        nc.gpsimd.affine_select(out=tri, in_=tri,
                                compare_op=mybir.AluOpType.is_ge,
                                fill=0.0, base=0, pattern=[[1, P]],
                                channel_multiplier=-1)
        eps_tile = consts.tile([P, 1], F32)
        nc.vector.memset(eps_tile, EPS)

        # Out-of-order pool lifetimes, closed manually:
        es_hT = ExitStack()       # phase A .. C   (left)
        es_vaug = ExitStack()     # phase B .. C   (left)
        es_attT = ExitStack()     # phase C .. D   (right)
        es_out1 = ExitStack()     # phase D .. E   (left)

        hTp = es_hT.enter_context(tc.tile_pool(name="hTp", bufs=1))
        hT = hTp.tile([P, ND, T], F16)

        # ---------------- Phase A: LN1 + transpose ----------------
        with tc.tile_pool(name="ln1", bufs=3) as lnp, \
             tc.tile_pool(name="pt_a", bufs=4, space="PSUM") as ptp:
            for tcn in range(NT):
                x_t = lnp.tile([P, D], F32, tag="x", name="x_t")
                nc.sync.dma_start(out=x_t, in_=x_d[tcn * P:(tcn + 1) * P, :])
                h_t = lnp.tile([P, D], F16, tag="h", name="h_t")
                _ln_chunk(nc, lnp, x_t, eps_tile, h_t)
                _transpose_into(nc, ptp, h_t, hT, tcn * P, identity)

        # ---------------- Phase B: v projection -> v_aug ----------------
        vaugp = es_vaug.enter_context(tc.tile_pool(name="vaugp", bufs=1))
        v_aug = vaugp.tile([P, NT, H, HS + 1], F16)
        with tc.tile_pool(name="wv", bufs=1) as wvp, \
             tc.tile_pool(name="ps_b", bufs=4, space="PSUM") as psb:
            wv_sb = wvp.tile([P, ND, D], F16)
            for dc in range(ND):
                nc.sync.dma_start(out=wv_sb[:, dc, :],
                                  in_=dram["Wv"][dc * P:(dc + 1) * P, :])
            bv_b = wvp.tile([P, D], F32)
            nc.sync.dma_start(out=bv_b,
                              in_=dram["bv"].ap().partition_broadcast(P))
            nc.vector.memset(v_aug[:, :, :, HS:HS + 1], 1.0)
            for tcn in range(NT):
                for ns in range(2):
                    pv = psb.tile([P, 512], F32, tag="pv", name="pv")
                    for dc in range(ND):
                        nc.tensor.matmul(pv,
                                         lhsT=hT[:, dc, tcn * P:(tcn + 1) * P],
                                         rhs=wv_sb[:, dc, ns * 512:(ns + 1) * 512],
                                         start=(dc == 0), stop=(dc == ND - 1))
                    bv_ap = bv_b[:, ns * 512:(ns + 1) * 512].rearrange(
                        "p (h s) -> p h s", s=HS)
                    nc.vector.tensor_add(
                        out=v_aug[:, tcn, ns * 8:(ns + 1) * 8, 0:HS],
                        in0=pv.rearrange("p (h s) -> p h s", s=HS),
                        in1=bv_ap)

        # ---------------- Phase C: attention per head-pair ----------------
        attTp = es_attT.enter_context(
            tc.tile_pool(name="attTp", bufs=1, side="right"))
        attT = attTp.tile([P, ND, T], F16)
        with tc.tile_pool(name="qk", bufs=2) as qkp, \
             tc.tile_pool(name="e", bufs=1) as ep, \
             tc.tile_pool(name="attn_sm", bufs=2) as smp, \
             tc.tile_pool(name="ps_q", bufs=2, space="PSUM") as psq, \
             tc.tile_pool(name="ps_s", bufs=2, space="PSUM") as pss, \
             tc.tile_pool(name="ps_av", bufs=1, space="PSUM") as psav:
            e_tiles = [ep.tile([P, E_TOT], F16, tag=f"e{i}", name=f"e{i}")
                       for i in range(2)]
            bq_sb = ep.tile([P, ND], F32, name="bq_sb")
            nc.sync.dma_start(out=bq_sb, in_=dram["bq"].ap())
            bk_sb = ep.tile([P, ND], F32, name="bk_sb")
            nc.sync.dma_start(out=bk_sb, in_=dram["bk"].ap())
            for m in range(ND):  # head pair m -> heads 2m, 2m+1
                wq_m = qkp.tile([P, ND, P], F16, tag="wqm", name="wq_m")
                nc.sync.dma_start(out=wq_m, in_=dram["Wq"][m])
                wk_m = qkp.tile([P, ND, P], F16, tag="wkm", name="wk_m")
                nc.sync.dma_start(out=wk_m, in_=dram["Wk"][m])
                qT_m = qkp.tile([P, T], F16, tag="qTm", name="qT_m")
                kT_m = qkp.tile([P, T], F16, tag="kTm", name="kT_m")
                for dst, w_m, b_sb in ((qT_m, wq_m, bq_sb),
                                       (kT_m, wk_m, bk_sb)):
                    for ns in range(2):
                        pq = psq.tile([P, 512], F32, tag="pq", name="pq")
                        for dc in range(ND):
                            nc.tensor.matmul(
                                pq, lhsT=w_m[:, dc, :],
                                rhs=hT[:, dc, ns * 512:(ns + 1) * 512],
                                start=(dc == 0), stop=(dc == ND - 1))
                        nc.vector.tensor_scalar_add(
                            out=dst[:, ns * 512:(ns + 1) * 512], in0=pq,
                            scalar1=b_sb[:, m:m + 1])

                # scores + exp (+ diagonal causal mask). Chunks of an
                # E_GROUP share one two-bank PSUM tile and one exp op over
                # the group's contiguous E span (ACT per-op overhead is
                # ~0.4us and is the phase bottleneck).
                for grp in E_GROUPS:
                    for hs_sel in range(2):
                        pb = hs_sel * HS
                        ps_ = pss.tile([P, 1024], F32, tag="ps", name="ps_")
                        goff = 0
                        for i in grp:
                            span = T - i * P
                            off = 0
                            while off < span:
                                qw = min(512, span - off)
                                nc.tensor.matmul(
                                    ps_[:, goff + off:goff + off + qw],
                                    lhsT=kT_m[pb:pb + HS, i * P:(i + 1) * P],
                                    rhs=qT_m[pb:pb + HS,
                                             i * P + off:i * P + off + qw],
                                    start=True, stop=True)
                                off += qw
                            goff += span
                        e0 = E_OFF[grp[0]]
                        ec = e_tiles[hs_sel][:, e0:e0 + goff]
                        nc.scalar.activation(out=ec, in_=ps_[:, 0:goff],
                                             func=AF.Exp, scale=SCALE)
                        for i in grp:
                            dg = e_tiles[hs_sel][:, E_OFF[i]:E_OFF[i] + P]
                            nc.vector.tensor_mul(out=dg, in0=dg, in1=tri)

                # AV + normalize; i-outer so both tq-slice accumulation
                # groups reuse the stationary v_aug block per chunk.
                for hs_sel in range(2):
                    h_glob = 2 * m + hs_sel
                    pavs = [psav.tile([HS + 1, 512], F32, tag=f"pav{s}",
                                      name=f"pav{s}") for s in range(2)]
                    for i in range(NT):
                        for s in range(2):
                            if i * P >= (s + 1) * 512:
                                continue
                            i_last = min(NT - 1, ((s + 1) * 512 - 1) // P)
                            sub_lo = max(i * P, s * 512)
                            width = (s + 1) * 512 - sub_lo
                            off = sub_lo - s * 512
                            e_ap = e_tiles[hs_sel][
                                :, E_OFF[i] + sub_lo - i * P:
                                E_OFF[i] + sub_lo - i * P + width]
                            nc.tensor.matmul(
                                pavs[s][:, off:off + width],
                                lhsT=v_aug[:, i, h_glob, :],
                                rhs=e_ap,
                                start=(i == 0), stop=(i == i_last))
                    for s in range(2):
                        pav = pavs[s]
                        recip = smp.tile([1, 512], F32, tag="recip",
                                         name="recip")
                        nc.vector.reciprocal(out=recip,
                                             in_=pav[HS:HS + 1, :])
                        bcast = smp.tile([HS, 512], F32, tag="bcast",
                                         name="bcast")
                        nc.gpsimd.partition_broadcast(out_ap=bcast,
                                                      in_ap=recip,
                                                      channels=HS)
                        p0 = hs_sel * HS
                        nc.vector.tensor_mul(
                            out=attT[p0:p0 + HS, m, s * 512:(s + 1) * 512],
                            in0=pav[0:HS, :], in1=bcast)
        es_vaug.close()
        es_hT.close()

        # ---------------- Phase D: Wo + residual + LN2 ----------------
        out1p = es_out1.enter_context(tc.tile_pool(name="out1p", bufs=1))
        out1 = out1p.tile([P, NT, D], F16)
        h2T = out1p.tile([P, ND, T], F16)
        with tc.tile_pool(name="wo", bufs=1) as wop, \
             tc.tile_pool(name="ln2", bufs=2) as ln2p, \
             tc.tile_pool(name="ps_d", bufs=4, space="PSUM") as psd, \
             tc.tile_pool(name="pt_d", bufs=4, space="PSUM") as ptd:
            bo_b = wop.tile([P, D], F32)
            nc.sync.dma_start(out=bo_b,
                              in_=dram["bo"].ap().partition_broadcast(P))
            wo_sb = wop.tile([P, ND, D], F16)
            for dc in range(ND):
                nc.sync.dma_start(out=wo_sb[:, dc, :],
                                  in_=dram["Wo"][dc * P:(dc + 1) * P, :])
            for tcn in range(NT):
                x_t = ln2p.tile([P, D], F32, tag="x2", name="x_t2")
                nc.sync.dma_start(out=x_t, in_=x_d[tcn * P:(tcn + 1) * P, :])
                for ns in range(2):
                    po = psd.tile([P, 512], F32, tag="po", name="po")
                    for dc in range(ND):
                        nc.tensor.matmul(
                            po, lhsT=attT[:, dc, tcn * P:(tcn + 1) * P],
                            rhs=wo_sb[:, dc, ns * 512:(ns + 1) * 512],
                            start=(dc == 0), stop=(dc == ND - 1))
                    stage = ln2p.tile([P, 512], F32, tag="stage", name="stage")
                    nc.vector.tensor_add(out=stage, in0=po,
                                         in1=x_t[:, ns * 512:(ns + 1) * 512])
                    nc.vector.tensor_add(
                        out=out1[:, tcn, ns * 512:(ns + 1) * 512],
                        in0=stage, in1=bo_b[:, ns * 512:(ns + 1) * 512])
                h2_t = ln2p.tile([P, D], F16, tag="h2", name="h2_t")
                _ln_chunk(nc, ln2p, out1[:, tcn, :], eps_tile, h2_t)
                _transpose_into(nc, ptd, h2_t, h2T, tcn * P, identity)
        es_attT.close()

        # ---------------- Phase E: FFN ----------------
        with tc.tile_pool(name="w2", bufs=1) as w2p, \
             tc.tile_pool(name="fTp", bufs=1) as fTp, \
             tc.tile_pool(name="w1", bufs=3) as w1p, \
             tc.tile_pool(name="ffn_out", bufs=3) as fop, \
             tc.tile_pool(name="ps_f", bufs=2, space="PSUM") as psf, \
             tc.tile_pool(name="ps_o2", bufs=2, space="PSUM") as pso:
            b1_sb = w2p.tile([P, NG], F32)
            nc.sync.dma_start(out=b1_sb, in_=dram["b1"].ap())
            b2_b = w2p.tile([P, D], F32)
            nc.sync.dma_start(out=b2_b,
                              in_=dram["b2"].ap().partition_broadcast(P))
            w2_sb = w2p.tile([P, NG, D], F16)
            for g in range(NG):
                nc.sync.dma_start(out=w2_sb[:, g, :],
                                  in_=dram["W2"][g * P:(g + 1) * P, :])
            fT = fTp.tile([P, NG, T], F16)
            for g in range(NG):
                w1_g = w1p.tile([P, ND, P], F16, tag="w1g", name="w1_g")
                nc.sync.dma_start(out=w1_g, in_=dram["W1"][g])
                for th in range(2):
                    pf = psf.tile([P, 512], F32, tag="pf", name="pf")
                    for dc in range(ND):
                        nc.tensor.matmul(
                            pf, lhsT=w1_g[:, dc, :],
                            rhs=h2T[:, dc, th * 512:(th + 1) * 512],
                            start=(dc == 0), stop=(dc == ND - 1))
                    nc.scalar.activation(
                        out=fT[:, g, th * 512:(th + 1) * 512], in_=pf,
                        func=AF.Relu, bias=b1_sb[:, g:g + 1], scale=1.0)
            for tcn in range(NT):
                for js in range(2):
                    po2 = pso.tile([P, 512], F32, tag="po2", name="po2")
                    for g in range(NG):
                        nc.tensor.matmul(
                            po2, lhsT=fT[:, g, tcn * P:(tcn + 1) * P],
                            rhs=w2_sb[:, g, js * 512:(js + 1) * 512],
                            start=(g == 0), stop=(g == NG - 1))
                    ofin = fop.tile([P, 512], F32, tag="ofin", name="ofin")
                    nc.vector.tensor_add(
                        out=ofin, in0=po2,
                        in1=out1[:, tcn, js * 512:(js + 1) * 512])
                    nc.vector.tensor_add(
                        out=ofin, in0=ofin,
                        in1=b2_b[:, js * 512:(js + 1) * 512])
                    nc.sync.dma_start(
                        out=out_d[tcn * P:(tcn + 1) * P,
                                  js * 512:(js + 1) * 512],
                        in_=ofin)
        es_out1.close()


_NC_CACHE = {}


def _get_nc():
    if "nc" not in _NC_CACHE:
        _NC_CACHE["nc"] = build(0)
    return _NC_CACHE["nc"]


def prepare_base_inputs(inputs):
    """Host-side input transform (shared by all cores).

    - Fold LN gains into the consuming weight matrices (Wq/Wk/Wv get
      ln1_g row-scaling; W1 gets ln2_g) and LN biases into bias rows
      (bq/bk/bv = ln1_b @ W; b1 += ln2_b @ W1). Exact algebra: with
      h = h0*g + b, h@W = h0@(g*W) + b@W.
    - Cast matmul weights to fp16.
    - Pre-pack Wq/Wk per head-pair column block and W1 per dff chunk so
      the device DMAs read 2KB-contiguous lines.
    """
    f32 = lambda k: np.asarray(inputs[k], np.float64)
    g1 = f32("ln1_g")[:, None]
    b1v = f32("ln1_b")
    g2 = f32("ln2_g")[:, None]
    b2v = f32("ln2_b")
    Wq, Wk, Wv, Wo = f32("Wq"), f32("Wk"), f32("Wv"), f32("Wo")
    W1, W2 = f32("W1"), f32("W2")

    def pack_cols(w, n_out_chunks):
        # [D, C*P] -> [C, P(row-sub), D//P(row-chunk), P(col)]
        d = w.shape[0]
        return np.ascontiguousarray(
            w.reshape(d // P, P, n_out_chunks, P).transpose(2, 1, 0, 3)
            .astype(np.float16))

    base = {
        "Wq": pack_cols(g1 * Wq, ND),
        "Wk": pack_cols(g1 * Wk, ND),
        "Wv": np.ascontiguousarray((g1 * Wv).astype(np.float16)),
        "Wo": np.ascontiguousarray(Wo.astype(np.float16)),
        "W1": pack_cols(g2 * W1, NG),
        "W2": np.ascontiguousarray(W2.astype(np.float16)),
        # [D] -> [P, chunks]: chunk c, sub-row p holds vec[c*P + p]
        "bq": np.ascontiguousarray(
            (b1v @ Wq).astype(np.float32).reshape(ND, P).T),
        "bk": np.ascontiguousarray(
            (b1v @ Wk).astype(np.float32).reshape(ND, P).T),
        "bv": (b1v @ Wv).astype(np.float32),
        "b1": np.ascontiguousarray(
            (f32("b1") + b2v @ W1).astype(np.float32).reshape(NG, P).T),
        "bo": np.asarray(inputs["bo"], np.float32),
        "b2": np.asarray(inputs["b2"], np.float32),
    }
    return base


def kernel(**inputs):
    from concourse.bass_utils import run_bass_kernel_spmd

    nc = _get_nc()
    x = np.ascontiguousarray(np.asarray(inputs["x"], dtype=np.float32))
    base = prepare_base_inputs(inputs)
    in_maps = [dict(x=np.ascontiguousarray(x[c]), **base)
               for c in range(N_CORES)]
    res = run_bass_kernel_spmd(nc, in_maps, list(range(N_CORES)))
    return np.stack([res.results[c]["out"] for c in range(N_CORES)], axis=0)
